# revision 1
# baseline (speedup 1.0000x reference)
"""Conv2d(256->256, 3x3, pad=1) on 8 TRN2 NeuronCores.

Sharding: data-parallel over output rows (H). Each core computes all 256
output channels for a 28-row slice of the output; the kernel (weights) are
replicated. This keeps the PE array fully loaded (M=128 output channels per
matmul) vs. out-channel sharding which would leave M=32.

Per core the conv is an implicit GEMM: out[o, h, w] = sum over (c, kh, kw) of
xpad[c, h+kh, w+kw] * k[o, c, kh, kw]. Contraction = 2 c-blocks x 9 taps = 18
accumulating matmuls per PSUM tile of [128 o, 2 h-rows x 224 w = 448].
Matmuls run in float32r (fp32 data streamed at bf16 rate — 4x faster than
fp32 matmul, ~1.4e-4 L2 rel err measured on HW vs fp64 at this contraction
depth; fp32 would be ~1.6e-7 but 4 cycles/row).

Measured on trn2 (8 cores): ~121.5-125 us HW exec (+-2 us run-to-run from
preamble/teardown jitter). Breakdown: ~7 us framework preamble, ~4.5 us DMA
gate (hidden behind PE warmup matmuls that keep the HAM clock-gate at 8/8 —
the gate is bound by ~0.7 us/instruction Sync descriptor generation plus the
early transfer rate, so the first pieces are tiny and ordered to match the
head schedule), ~105 us dense matmul stream starting at ~11.5 us (504 MMs,
~205 ns cadence; LDWEIGHTS-for-f32r at ~190 ns is the co-bottleneck and is
unavoidable — f32r matmuls must self-load weights, standalone LDWEIGHTS
returns zeros for f32r), ~5 us drain/teardown. The first three row-bands are
split into c-block halves (six b0-only half-groups across six PSUM banks,
then the b1 halves in DMA-arrival order) so the PE runs gapless from the
moment the first ~0.4 MB lands.
"""

import sys

sys.path.insert(0, "/opt/trn_rl_repo")

import numpy as np

import concourse.mybir as mybir
from concourse import bacc
from concourse.tile import TileContext
from concourse.bass_utils import run_bass_kernel_spmd

N_CORES = 8
C, H, W = 256, 224, 224
O = 256
KH = KW = 3
HS = H // N_CORES          # 28 output rows per core
HROWS = 2                  # output rows per PSUM tile (N = 2*224 = 448)
CB = C // 128              # c blocks
OB = O // 128              # o blocks

_CACHE = {}
LAST_RESULTS = None        # test.py reads exec_time_ns / trace path from here
TRACE = False


def _build():
    nc = bacc.Bacc(None, target_bir_lowering=False)

    xs = nc.dram_tensor(
        "xs", [CB, 128, HS + 2, W + 2], mybir.dt.float32r, kind="ExternalInput"
    )
    w = nc.dram_tensor(
        "w", [CB, OB, 128, KH * KW, 128], mybir.dt.float32r, kind="ExternalInput"
    )
    out = nc.dram_tensor(
        "out", [OB, 128, HS, W], mybir.dt.float32, kind="ExternalOutput"
    )

    n_warm = 18
    with TileContext(nc) as tc:
        with (
            tc.tile_pool(name="warm", bufs=1) as pwarm,
            tc.tile_pool(name="win", bufs=1) as pw,
            tc.tile_pool(name="xin", bufs=1) as px,
            tc.tile_pool(name="psumw", bufs=1, space="PSUM") as ppw,
            tc.tile_pool(name="psum", bufs=7, space="PSUM") as pp,
            tc.tile_pool(name="outp", bufs=4) as po,
        ):
            # PE warmup: dummy matmuls on a memset tile while input DMAs
            # stream, so the HAM clock-gate is at 8/8 when real work starts.
            wt0 = pwarm.tile([128, 256], mybir.dt.bfloat16, tag="warm")
            ps0 = ppw.tile([128, 256], mybir.dt.float32, tag="warmps")
            nc.vector.memset(wt0[:], 0.0)
            for _ in range(n_warm):
                nc.tensor.matmul(ps0[:], wt0[:, :128], wt0[:], start=True, stop=True)

            # One big x tile per c-block, filled by 2-row chunked DMAs so the
            # first matmuls only wait on the first rows, not the whole tile.
            x_sb = [
                px.tile(
                    [128, HS + 2, W + 2], mybir.dt.float32r, tag=f"x{b}", name=f"x{b}"
                )
                for b in range(CB)
            ]
            w_sb = [
                pw.tile(
                    [128, KH * KW, O], mybir.dt.float32r, tag=f"w{b}", name=f"w{b}"
                )
                for b in range(CB)
            ]
            # Gate DMAs in exact consumption order of the c-block-split head
            # schedule below: b0 pieces (both ob weight halves) first so four
            # half-groups of b0 work can run while b1's pieces stream in.
            def dma_w(b, ob):
                nc.sync.dma_start(
                    out=w_sb[b][:, :, ob * 128 : (ob + 1) * 128], in_=w[b, ob]
                )

            def dma_x(b, r0, r1):
                nc.sync.dma_start(
                    out=x_sb[b][:, r0:r1, :], in_=xs[b, :, r0:r1, :]
                )

            # First weight quarter split at tap granularity so the very first
            # matmuls gate on ~0.4 MB instead of ~1 MB; x rows in 2-row pieces
            # ordered to match the head schedule's consumption order.
            nc.sync.dma_start(out=w_sb[0][:, 0:3, 0:128], in_=w[0, 0, :, 0:3, :])
            dma_x(0, 0, 2)
            nc.sync.dma_start(out=w_sb[0][:, 3:6, 0:128], in_=w[0, 0, :, 3:6, :])
            dma_x(0, 2, 4)
            nc.sync.dma_start(out=w_sb[0][:, 6:9, 0:128], in_=w[0, 0, :, 6:9, :])
            dma_x(0, 4, 6)
            dma_x(0, 6, 8)
            dma_w(0, 1)
            dma_x(1, 0, 2)
            dma_x(1, 2, 4)
            dma_x(1, 4, 6)
            dma_w(1, 0)
            dma_x(1, 6, 8)
            dma_w(1, 1)
            for r in range(8, HS + 2, 2):
                for b in range(CB):
                    dma_x(b, r, r + 2)

            n_acc = CB * KH * KW

            def mm_group(ps, h0, ob, bs, first, last):
                idx = 0
                for b in bs:
                    for k in range(KH * KW):
                        kh, kw = divmod(k, KW)
                        nc.tensor.matmul(
                            ps[:],
                            w_sb[b][:, k, ob * 128 : (ob + 1) * 128],
                            x_sb[b][:, h0 + kh : h0 + kh + HROWS, kw : kw + W],
                            start=(first and idx == 0),
                            stop=(last and idx == len(bs) * KH * KW - 1),
                        )
                        idx += 1

            def finish_group(ps, h0, ob):
                ot = po.tile([128, HROWS, W], mybir.dt.float32, tag="ot", name="ot")
                nc.vector.tensor_copy(out=ot[:], in_=ps[:])
                nc.sync.dma_start(out=out[ob, :, h0 : h0 + HROWS, :], in_=ot[:])

            # First three bands: run the b=0 halves of six groups (3 bands x
            # 2 ob) while b=1's weights/rows are still in flight, then add
            # the b=1 halves in the same arrival order. Keeps the PE dense
            # from the moment the first ~0.4 MB lands.
            head = [(0, 0), (2, 0), (4, 0), (0, 1), (2, 1), (4, 1)]
            ps_head = {}
            for h0, ob in head:
                ps = pp.tile([128, HROWS, W], mybir.dt.float32, tag="ps", name="ps")
                ps_head[(h0, ob)] = ps
                mm_group(ps, h0, ob, [0], first=True, last=False)
            for h0, ob in [(0, 0), (0, 1), (2, 0), (2, 1), (4, 0), (4, 1)]:
                ps = ps_head[(h0, ob)]
                mm_group(ps, h0, ob, [1], first=False, last=True)
                finish_group(ps, h0, ob)

            for h0 in range(3 * HROWS, HS, HROWS):
                for ob in range(OB):
                    ps = pp.tile([128, HROWS, W], mybir.dt.float32, tag="ps", name="ps")
                    mm_group(ps, h0, ob, list(range(CB)), first=True, last=True)
                    finish_group(ps, h0, ob)

    nc.compile()
    return nc


def kernel(x: np.ndarray, kernel: np.ndarray) -> np.ndarray:
    global LAST_RESULTS
    if "nc" not in _CACHE:
        _CACHE["nc"] = _build()
    nc = _CACHE["nc"]

    x = np.ascontiguousarray(x, dtype=np.float32)
    kw_arr = np.ascontiguousarray(kernel, dtype=np.float32)

    xp = np.pad(x, ((0, 0), (1, 1), (1, 1)))          # [C, H+2, W+2]
    xp = xp.reshape(CB, 128, H + 2, W + 2)
    # w_t[b, ob, p, k, oc] = kernel[ob*128+oc, b*128+p, kh, kw] — each (b, ob)
    # quarter is contiguous per partition for a clean DMA line.
    w_t = np.ascontiguousarray(
        kw_arr.transpose(1, 2, 3, 0)
        .reshape(CB, 128, KH * KW, OB, 128)
        .transpose(0, 3, 1, 2, 4)
    )

    in_maps = []
    for i in range(N_CORES):
        xs_i = np.ascontiguousarray(xp[:, :, i * HS : i * HS + HS + 2, :])
        in_maps.append({"xs": xs_i, "w": w_t})

    # The axon-tunneled device occasionally wedges with a transient
    # NRT_EXEC_UNIT_UNRECOVERABLE; a retry on a fresh execute recovers it.
    last_err = None
    for _ in range(3):
        try:
            results = run_bass_kernel_spmd(
                nc, in_maps, core_ids=list(range(N_CORES)), trace=TRACE
            )
            break
        except Exception as e:  # noqa: BLE001
            last_err = e
    else:
        raise last_err
    LAST_RESULTS = results

    parts = [r["out"].reshape(O, HS, W) for r in results.results]
    return np.concatenate(parts, axis=1)



# revision 3
# speedup vs baseline: 1.3803x; 1.3803x over previous
"""Conv2d(256->256, 3x3, pad=1) on 8 TRN2 NeuronCores — Winograd F(2,3) along W.

Sharding: data-parallel over output rows (H), 28 rows/core, weights replicated
(kept from the direct-conv baseline: it keeps M=128 output channels per matmul).

Algorithm: 1D Winograd F(2,3) on the W axis, direct convolution on H (3 taps)
and channels. Per output-tile column pair the direct conv needs 9 taps x 2
c-blocks = 18 accumulation columns/output; Winograd needs 4 points/2 outputs x
3 kh x 2 cb = 12 — a 1.5x reduction in PE work (504 -> 336 matmuls of N=448).

  z0 = d0 - d2, z1 = d1 + d2, z2 = d2 - d1, z3 = d1 - d3   (input, DVE, bf16)
  Gg = [g0, (g0+g1+g2)/2, (g0-g1+g2)/2, g2]                 (weights, host)
  m_j[o,h,t] = sum_{c,kh} z_j[c,h+kh,t] * Gg_j[o,c,kh]      (PE, PSUM fp32)
  out[2t]   = m0 + m1 + m2                                  (output, DVE)
  out[2t+1] = m1 - m2 - m3

Everything on the PE is bf16 (x and Gg rounded on host / by DVE), which also
lets LDWEIGHTS pipeline ahead of matmuls (f32r must self-load weights).
"""

import sys

sys.path.insert(0, "/opt/trn_rl_repo")

import numpy as np
import ml_dtypes

import concourse.mybir as mybir
from concourse import bacc
from concourse.tile import TileContext
from concourse.bass_utils import run_bass_kernel_spmd

N_CORES = 8
C, H, W = 256, 224, 224
O = 256
KH = 3
HS = H // N_CORES          # 28 output rows per core
HR = 4                     # output rows per PSUM tile group
HG = HS // HR              # 7 h-groups
T = W // 2                 # 112 Winograd tiles per row
J = 4                      # Winograd points per tile
CB = C // 128
OB = O // 128
WP = W + 2                 # padded row width
HP = HS + 2                # x rows per core (1 halo each side)

_CACHE = {}
LAST_RESULTS = None
TRACE = False

BF16 = mybir.dt.bfloat16
F32 = mybir.dt.float32


def _build():
    nc = bacc.Bacc(None, target_bir_lowering=False)

    xs = nc.dram_tensor("xs", [CB, 128, HP, WP], BF16, kind="ExternalInput")
    # w[cb, ob, c, j*3+kh, o]
    w = nc.dram_tensor("w", [CB, OB, 128, J * KH, 128], BF16, kind="ExternalInput")
    out = nc.dram_tensor("out", [OB, 128, HS, W], F32, kind="ExternalOutput")

    n_warm = 20
    add = mybir.AluOpType.add
    sub = mybir.AluOpType.subtract

    with TileContext(nc) as tc:
        with (
            tc.tile_pool(name="warm", bufs=1) as pwarm,
            tc.tile_pool(name="win", bufs=1) as pw,
            tc.tile_pool(name="xin", bufs=1) as px,
            tc.tile_pool(name="zbuf", bufs=1) as pz,
            tc.tile_pool(name="psumw", bufs=1, space="PSUM") as ppw,
            tc.tile_pool(name="psum", bufs=7, space="PSUM") as pp,
            tc.tile_pool(name="tmp", bufs=4) as pt,
            tc.tile_pool(name="outp", bufs=4) as po,
        ):
            # PE warmup against the HAM clock-gate while input DMAs stream.
            wt0 = pwarm.tile([128, 256], BF16, tag="warm")
            ps0 = ppw.tile([128, 256], F32, tag="warmps")
            nc.vector.memset(wt0[:], 0.0)
            for _ in range(n_warm):
                nc.tensor.matmul(ps0[:], wt0[:, :128], wt0[:], start=True, stop=True)

            x_sb = [
                px.tile([128, HP, WP], BF16, tag=f"x{b}", name=f"x{b}")
                for b in range(CB)
            ]
            w_sb = [
                pw.tile([128, J * KH, O], BF16, tag=f"w{b}", name=f"w{b}")
                for b in range(CB)
            ]
            z_sb = [
                pz.tile([128, J, HP, T], BF16, tag=f"z{b}", name=f"z{b}")
                for b in range(CB)
            ]

            def dma_w(b, ob, j0, j1):
                nc.sync.dma_start(
                    out=w_sb[b][:, j0 * KH : j1 * KH, ob * 128 : (ob + 1) * 128],
                    in_=w[b, ob, :, j0 * KH : j1 * KH, :],
                )

            def dma_x(b, r0, r1):
                nc.sync.dma_start(out=x_sb[b][:, r0:r1, :], in_=xs[b, :, r0:r1, :])

            # Head: j0 weights for ob0 + first x rows, ordered so the first
            # matmul group's deps land first.
            dma_w(0, 0, 0, 1)
            dma_x(0, 0, 3)
            dma_w(1, 0, 0, 1)
            dma_x(1, 0, 3)
            dma_w(0, 0, 1, 4)
            dma_x(0, 3, 6)
            dma_w(1, 0, 1, 4)
            dma_x(1, 3, 6)
            dma_w(0, 1, 0, 4)
            dma_w(1, 1, 0, 4)
            for r in range(6, HP, 4):
                r1 = min(r + 4, HP)
                dma_x(0, r, r1)
                dma_x(1, r, r1)

            # Input transform: z row r depends only on x row r.
            # z0 = d0-d2, z1 = d1+d2, z2 = d2-d1, z3 = d1-d3 with d_k = x[2t+k].
            def ztrans(b, r0, r1):
                x_ = x_sb[b]
                z_ = z_sb[b]
                d = [x_[:, r0:r1, k : k + 223 : 2] for k in range(4)]
                nc.vector.tensor_tensor(z_[:, 0, r0:r1, :], d[0], d[2], sub)
                nc.vector.tensor_tensor(z_[:, 1, r0:r1, :], d[1], d[2], add)
                nc.vector.tensor_tensor(z_[:, 2, r0:r1, :], d[2], d[1], sub)
                nc.vector.tensor_tensor(z_[:, 3, r0:r1, :], d[1], d[3], sub)

            ztrans(0, 0, 6)
            ztrans(1, 0, 6)

            def mm_group(ps_j, h0, ob):
                # 4 psum tiles (one per Winograd point), 6 accumulating
                # matmuls each (cb-major so the head can start on cb0).
                for j in range(J):
                    idx = 0
                    for b in range(CB):
                        for kh in range(KH):
                            nc.tensor.matmul(
                                ps_j[j][:],
                                w_sb[b][:, j * KH + kh, ob * 128 : (ob + 1) * 128],
                                z_sb[b][:, j, h0 + kh : h0 + kh + HR, :],
                                start=(idx == 0),
                                stop=(idx == CB * KH - 1),
                            )
                            idx += 1

            def finish_group(ps_j, h0, ob):
                # out[2t] = m0+m1+m2, out[2t+1] = m1-m2-m3; tensor_tensor
                # can't read two PSUM operands, so ACT stages m1/m2 in SBUF.
                ot = po.tile([128, HR, W], F32, tag="ot", name="ot")
                s1 = pt.tile([128, HR, T], F32, tag="s1", name="s1")
                s2 = pt.tile([128, HR, T], F32, tag="s2", name="s2")
                t0 = pt.tile([128, HR, T], F32, tag="t0", name="t0")
                t1 = pt.tile([128, HR, T], F32, tag="t1", name="t1")
                nc.scalar.copy(s1[:], ps_j[1][:])
                nc.scalar.copy(s2[:], ps_j[2][:])
                nc.vector.tensor_tensor(t0[:], ps_j[0][:], s1[:], add)
                nc.vector.tensor_tensor(ot[:, :, 0:223:2], t0[:], s2[:], add)
                nc.vector.tensor_tensor(t1[:], s1[:], s2[:], sub)
                nc.vector.tensor_tensor(ot[:, :, 1:224:2], t1[:], ps_j[3][:], sub)
                nc.scalar.dma_start(out=out[ob, :, h0 : h0 + HR, :], in_=ot[:])

            for hg in range(HG):
                h0 = hg * HR
                if 0 < hg:
                    r0 = 4 * hg + 2
                    r1 = min(r0 + 4, HP)
                    if r0 < r1:
                        ztrans(0, r0, r1)
                        ztrans(1, r0, r1)
                for ob in range(OB):
                    ps_j = [
                        pp.tile([128, HR, T], F32, tag="ps", name="ps")
                        for _ in range(J)
                    ]
                    mm_group(ps_j, h0, ob)
                    finish_group(ps_j, h0, ob)

    nc.compile()
    return nc


def kernel(x: np.ndarray, kernel: np.ndarray) -> np.ndarray:
    global LAST_RESULTS
    if "nc" not in _CACHE:
        _CACHE["nc"] = _build()
    nc = _CACHE["nc"]

    x = np.ascontiguousarray(x, dtype=np.float32)
    kw_arr = np.ascontiguousarray(kernel, dtype=np.float32)

    xp = np.pad(x, ((0, 0), (1, 1), (1, 1)))          # [C, H+2, W+2]

    # Winograd weight transform along kw: Gg[o,c,kh,j]
    g = kw_arr  # [O, C, 3, 3]
    gg = np.empty((O, C, KH, J), dtype=np.float32)
    gg[..., 0] = g[..., 0]
    gg[..., 1] = 0.5 * (g[..., 0] + g[..., 1] + g[..., 2])
    gg[..., 2] = 0.5 * (g[..., 0] - g[..., 1] + g[..., 2])
    gg[..., 3] = g[..., 2]
    # w_t[cb, ob, c, j*3+kh, o]
    w_t = np.ascontiguousarray(
        gg.reshape(OB, 128, CB, 128, KH, J)
        .transpose(2, 0, 3, 5, 4, 1)
        .reshape(CB, OB, 128, J * KH, 128)
        .astype(ml_dtypes.bfloat16)
    )

    in_maps = []
    for i in range(N_CORES):
        xs_i = np.ascontiguousarray(
            xp[:, i * HS : i * HS + HP, :].reshape(CB, 128, HP, WP).astype(
                ml_dtypes.bfloat16
            )
        )
        in_maps.append({"xs": xs_i, "w": w_t})

    # The axon-tunneled device occasionally wedges with a transient
    # NRT_EXEC_UNIT_UNRECOVERABLE; a retry on a fresh execute recovers it.
    last_err = None
    for _ in range(3):
        try:
            results = run_bass_kernel_spmd(
                nc, in_maps, core_ids=list(range(N_CORES)), trace=TRACE
            )
            break
        except Exception as e:  # noqa: BLE001
            last_err = e
    else:
        raise last_err
    LAST_RESULTS = results

    parts = [r["out"].reshape(O, HS, W) for r in results.results]
    return np.concatenate(parts, axis=1)
